# revision 76
# baseline (speedup 1.0000x reference)
"""Tensor-parallel multi-head attention kernel for 8 Trainium2 NeuronCores.

Sharding: tensor-parallel over heads. Each core owns 2 of the 16 heads
(a 128-dim slice of the projections). Wq/Wk/Wv are sharded column-wise
(output dim), Wo row-wise (input dim). Each core computes a full-shape
partial of the output projection; the host sums the 8 partials (the
"all-reduce") and transposes back. No device collectives are needed.

v2 design vs baseline:
 - Emission order per attention iteration is exp -> next-QK -> fillers
   -> AV, so filler matmuls are not queued behind the exp-dependent AV
   (the in-order PE otherwise idles ~1us/iteration on the sem wait).
 - ScalarE carries only the softmax exp plus the rms Ln/Exp rsqrt; the
   Square moved to DVE and the per-head mean-of-squares reduction +
   broadcast to gpsimd partition_all_reduce (result lands broadcast, so
   no ssq matmul, no broadcast matmul, no extra PSUM).
 - Softmax normalization: reciprocal_approx_fast on the sum rows, gpsimd
   partition_broadcast, DVE muls; AV psum freed early via an f32 copy so
   the next query-chunk's AV accumulation starts ~1.4us after the last.
 - PSUM rings by tag: scores 2x[128,1024] (4 banks), av 2, proj 1, op 1.
 - Fillers are micro-thunks (~1 PE matmul each) drained 2-3/iteration.
"""

import sys

if "/opt/trn_rl_repo" not in sys.path:
    sys.path.insert(0, "/opt/trn_rl_repo")

import numpy as np
import ml_dtypes

BF16 = ml_dtypes.bfloat16

B, T, C = 2, 2048, 1024
H, HD = 16, 64
BT = B * T            # 4096 tokens total
NCORES = 8
DPC = C // NCORES     # 128 projection dims per core (2 heads x 64)
NKT = T // 128        # 16 k-tiles of 128 tokens per batch
NCI = C // 128        # 8 contraction tiles for the projections
SCALE = 1.0 / 8.0     # 1/sqrt(HD)
EPS = float(np.finfo(np.float32).eps)
JQ = 512              # query-chunk width in the attention loop
NJQ = T // JQ         # 4 query chunks per batch
GR = 512              # projection granule (tokens)
NG = T // GR          # granules per batch per tensor
NEG = -30000.0        # additive mask bias for padded keys

_CACHE = {}


def _build_bass():
    import concourse.bass as bass
    from concourse import bacc, mybir, tile, bass_isa
    from concourse.masks import make_identity
    from contextlib import ExitStack

    dt = mybir.dt
    AF = mybir.ActivationFunctionType
    ts = bass.ts

    # Force all activations onto natural_log_exp_and_others (it contains
    # exp/ln/copy), so the kernel needs exactly one ACT table load.
    from concourse import bacc as _bacc_mod, hw_specs as _hw

    _orig_tables = _hw.get_activation_tables

    def _only_nl_exp(arch):
        t = _orig_tables(arch)
        return {
            name: (fns if name == "natural_log_exp_and_others" else set())
            for name, fns in t.items()
        }

    _bacc_mod.get_activation_tables = _only_nl_exp

    nc = bacc.Bacc("TRN2", target_bir_lowering=False, debug=False)

    xt_ext = nc.dram_tensor("xt", [128, NCI * BT], dt.bfloat16, kind="ExternalInput")
    wq_ext = nc.dram_tensor("wq", [128, NCI * DPC], dt.bfloat16, kind="ExternalInput")
    wk_ext = nc.dram_tensor("wk", [128, NCI * DPC], dt.bfloat16, kind="ExternalInput")
    wv_ext = nc.dram_tensor("wv", [128, NCI * DPC], dt.bfloat16, kind="ExternalInput")
    wo_ext = nc.dram_tensor("wo", [DPC, C], dt.bfloat16, kind="ExternalInput")
    bias_ext = nc.dram_tensor("bias", [128, B * NKT], dt.float32, kind="ExternalInput")
    out_ext = nc.dram_tensor("out", [128, NCI * BT], dt.bfloat16, kind="ExternalOutput")

    with ExitStack() as ctx:
        tc = ctx.enter_context(tile.TileContext(nc))
        singles = ctx.enter_context(tc.tile_pool(name="singles", bufs=1))
        work = ctx.enter_context(tc.tile_pool(name="work", bufs=1))
        se_pool = ctx.enter_context(tc.tile_pool(name="se", bufs=6))
        pp = ctx.enter_context(tc.tile_pool(name="pp", bufs=2, space="PSUM"))
        psp = ctx.enter_context(tc.tile_pool(name="psp", bufs=1, space="PSUM"))

        # ---- persistent SBUF state ----
        xt_sb = singles.tile([128, NCI * BT], dt.bfloat16)     # xT, ci-major
        wq_sb = singles.tile([128, NCI * DPC], dt.bfloat16)
        wk_sb = singles.tile([128, NCI * DPC], dt.bfloat16)
        wv_sb = singles.tile([128, NCI * DPC], dt.bfloat16)
        wo_sb = singles.tile([128, C], dt.bfloat16)
        bias_sb = singles.tile([128, B * NKT], dt.float32)
        qn_sb = singles.tile([128, BT], dt.bfloat16)           # rms-normed qT
        kn_sb = singles.tile([128, BT], dt.bfloat16)           # rms-normed kT
        # v tiles per (head, batch, k-tile): [v | ones] -> M=65 PV matmuls
        vext = singles.tile([128, 2, B, NKT, HD + 1], dt.bfloat16)
        yn_sb = singles.tile([128, BT], dt.bfloat16)           # normalized yT
        ident = singles.tile([128, 128], dt.bfloat16)
        eps_sb = singles.tile([128, 1], dt.float32)
        ones2 = singles.tile([128, 2], dt.bfloat16)  # per-head contraction masks

        # Input order matters: the first granule (k of batch 0, tokens
        # 0..511) needs wk + the first xt chunk; split chunks across queues
        # so they stream in parallel.
        WKH = NCI * DPC // 4
        for piece in range(4):
            nc.sync.dma_start(
                out=wk_sb[:, piece * WKH : (piece + 1) * WKH],
                in_=wk_ext.ap()[:, piece * WKH : (piece + 1) * WKH],
            )
        NTG = BT // GR
        w = NCI * GR
        h = w // 2

        def xt_chunk(tg, pieces=2):
            ph = w // pieces
            for piece in range(pieces):
                lo = tg * w + piece * ph
                nc.sync.dma_start(
                    out=xt_sb[:, lo : lo + ph], in_=xt_ext.ap()[:, lo : lo + ph]
                )

        xt_chunk(0, pieces=4)
        nc.sync.dma_start(out=wq_sb[:], in_=wq_ext.ap())
        nc.sync.dma_start(out=wv_sb[:], in_=wv_ext.ap())
        xt_chunk(1, pieces=4)
        nc.sync.dma_start(out=bias_sb[:], in_=bias_ext.ap())
        for tg in range(2, NTG):
            xt_chunk(tg)
        nc.sync.dma_start(out=wo_sb[:], in_=wo_ext.ap())

        nc.gpsimd.memset(eps_sb[:], EPS)
        nc.gpsimd.memset(vext[:, :, :, :, HD : HD + 1], 1.0)
        nc.gpsimd.memset(ones2[:], 0.0)
        nc.gpsimd.memset(ones2[0:64, 0:1], 1.0)
        nc.gpsimd.memset(ones2[64:128, 1:2], 1.0)
        make_identity(nc, ident[:])

        # ------------------------------------------------------------------
        # Granule builders. Each returns a list of micro-thunks (emission
        # order within the list must be preserved; roughly one PE matmul or
        # one DVE/gpsimd/scalar step each).
        # ------------------------------------------------------------------

        TAG_BUFS = {"proj": 1, "op": 1, "av": 2}

        def proj_mm_thunks(w_sb, cell, t0, tag):
            """All 8 accumulating matmuls as ONE thunk: the PSUM accumulation
            group must not interleave with other matmul groups."""

            def f():
                cell["ps"] = psp.tile(
                    [128, GR], dt.float32, tag=tag, bufs=TAG_BUFS[tag], name="ps"
                )
                tg = t0 // GR
                for ci in range(NCI):
                    c0 = tg * NCI * GR + ci * GR
                    nc.tensor.matmul(
                        cell["ps"][:],
                        lhsT=w_sb[:, ts(ci, DPC)],
                        rhs=xt_sb[:, c0 : c0 + GR],
                        start=(ci == 0),
                        stop=(ci == NCI - 1),
                    )

            return [(8, f)]

        def rms_granule_thunks(w_sb, dst_sb, b, g, tag, sib):
            """Project + rms-normalize GR tokens.

            Returns (body, tails): tails are (slot_offset, entry) deferred
            into following queue items so the scalar Ln/Exp and the
            broadcast chain don't head-of-line-block their engine FIFOs.
            ps lives in psum ring `tag`; the transient ssq in ring `sib`.
            """
            t0 = b * T + g * GR
            cell = {}
            body = proj_mm_thunks(w_sb, cell, t0, tag)

            def f_q2():
                q2b = work.tile([128, GR], dt.bfloat16, tag="q2b", bufs=2)
                q2s = work.tile([128, GR], dt.bfloat16, tag="q2s", bufs=2)
                nc.vector.tensor_copy(q2b[:], cell["ps"][:])
                nc.vector.tensor_mul(q2s[:], q2b[:], q2b[:])
                cell["q2s"] = q2s

            def f_ss():
                # per-head sum of squares via ones-matmul; rsqrt via Ln/Exp
                # (both live in the natural_log_exp table set)
                ssq = psp.tile(
                    [2, GR], dt.float32, tag=sib, bufs=TAG_BUFS[sib], name="ssq"
                )
                nc.tensor.matmul(
                    ssq[:], lhsT=ones2[:], rhs=cell["q2s"][:], start=True, stop=True
                )
                lnt2 = work.tile([2, GR], dt.float32, tag="lnt2", bufs=3)
                rinv2 = work.tile([2, GR], dt.float32, tag="rinv2", bufs=3)
                nc.scalar.activation(
                    out=lnt2[:],
                    in_=ssq[:],
                    func=AF.Ln,
                    bias=eps_sb[0:2, :],
                    scale=1.0 / HD,
                )
                nc.scalar.activation(
                    out=rinv2[:], in_=lnt2[:], func=AF.Exp, bias=0.0, scale=-0.5
                )
                cell["rinv2"] = rinv2

            body.append((1, f_q2))
            body.append((1, f_ss))

            def f_bc():
                # broadcast rinv per head via gpsimd (reads absolute
                # partition 0; head B round-trips base-0 staging + DMA shift)
                rinv2 = cell["rinv2"]
                rinvB0 = work.tile([1, GR], dt.float32, tag="rinvB0", bufs=3)
                nc.sync.dma_start(out=rinvB0[0:1, :], in_=rinv2[1:2, :])
                rbc = work.tile([128, GR], dt.float32, tag="rbc", bufs=3)
                rbB0 = work.tile([64, GR], dt.float32, tag="rbB0", bufs=3)
                nc.gpsimd.partition_broadcast(rbc[0:64, :], rinv2[0:1, :], 64)
                nc.gpsimd.partition_broadcast(rbB0[:], rinvB0[0:1, :], 64)
                nc.sync.dma_start(out=rbc[64:128, :], in_=rbB0[:])
                cell["rbc"] = rbc

            def f_fin():
                nc.vector.tensor_mul(
                    dst_sb[:, t0 : t0 + GR], cell["ps"][:], cell["rbc"][:]
                )
                import os

                if (
                    bool(int(os.environ.get("BASS_ATTN_DEBUG", "0")))
                    and dst_sb is qn_sb
                    and t0 == 0
                ):
                    for nm, src, shape, dty in [
                        ("dbg_q2s", cell["q2s"], [128, GR], dt.bfloat16),
                        ("dbg_msb", cell["rbc"], [128, GR], dt.float32),
                    ]:
                        extd = nc.dram_tensor(nm, shape, dty, kind="ExternalOutput")
                        nc.sync.dma_start(out=extd.ap(), in_=src[:])

            return body, [(3, (0, f_bc)), (6, (1, f_fin))]

        def v_granule_thunks(b, g, tag):
            """Project GR tokens of v, transpose 128-blocks into vext."""
            t0 = b * T + g * GR
            cell = {"dbg": b == 0 and g == 0}
            th = proj_mm_thunks(wv_sb, cell, t0, tag)

            def f_vt():
                vt = work.tile([128, GR], dt.bfloat16, tag="vt", bufs=2)
                nc.vector.tensor_copy(vt[:], cell["ps"][:])
                cell["vt"] = vt

            th.append((1, f_vt))

            def mk_tr(j):
                def f():
                    # SBUF->SBUF xbar transpose into a contiguous staging
                    # tile, then DVE copies into the two vext head slots
                    kt = g * (GR // 128) + j
                    vtT = work.tile([128, 128], dt.bfloat16, tag="vtT", bufs=3)
                    nc.sync.dma_start_transpose(
                        out=vtT[:], in_=cell["vt"][:, ts(j, 128)]
                    )
                    nc.vector.tensor_copy(vext[:, 0, b, kt, 0:HD], vtT[:, 0:HD])
                    nc.vector.tensor_copy(
                        vext[:, 1, b, kt, 0:HD], vtT[:, HD : 2 * HD]
                    )
                    if cell.get("dbg"):
                        import os

                        if bool(int(os.environ.get("BASS_ATTN_DEBUG", "0"))) and (
                            j == 0
                        ):
                            for nm, src in [
                                ("dbg_vt", cell["vt"]),
                                ("dbg_vtT", vtT),
                            ]:
                                extd = nc.dram_tensor(
                                    nm, list(src[:].shape), dt.bfloat16,
                                    kind="ExternalOutput",
                                )
                                nc.sync.dma_start(out=extd.ap(), in_=src[:])

                return f

            for j in range(GR // 128):
                th.append((1, mk_tr(j)))
            return th

        def outproj_chunk_thunks(b, ch):
            """8 d-tiles of the output projection for one 512-token chunk.

            Alternates psum tags op/proj so tile d+1's matmul does not wait
            on tile d's DVE cast.
            """
            t0 = b * T + ch * 512
            cell = {}
            th = []

            def mk(dtile):
                def f():
                    if dtile == 0:
                        cell["ob"] = work.tile(
                            [128, NCI, 512], dt.bfloat16, tag="ob", bufs=2, name="ob"
                        )
                    tag = "op" if dtile % 2 == 0 else "proj"
                    ps_o = psp.tile(
                        [128, 512],
                        dt.float32,
                        tag=tag,
                        bufs=TAG_BUFS[tag],
                        name="ps_o",
                    )
                    nc.tensor.matmul(
                        ps_o[:],
                        lhsT=wo_sb[:, ts(dtile, 128)],
                        rhs=yn_sb[:, t0 : t0 + 512],
                        start=True,
                        stop=True,
                    )
                    nc.vector.tensor_copy(cell["ob"][:, dtile, :], ps_o[:])

                return f

            for dtile in range(NCI):
                th.append((1, mk(dtile)))

            def f_dma():
                # chunk-major out layout: [128, chunk(8), NCI*512]; each
                # chunk's store is one contiguous 8KB run per partition
                ci = b * NJQ + ch
                W = NCI * 512
                dst = out_ext.ap()[:, ci * W : (ci + 1) * W]
                nc.sync.dma_start(
                    out=dst, in_=cell["ob"][:].rearrange("p a b -> p (a b)")
                )
                import os

                if (
                    bool(int(os.environ.get("BASS_ATTN_DEBUG", "0")))
                    and b == 1
                    and ch == 1
                ):
                    extd = nc.dram_tensor(
                        "dbg_ob", [128, NCI * 512], dt.bfloat16, kind="ExternalOutput"
                    )
                    nc.sync.dma_start(
                        out=extd.ap(),
                        in_=cell["ob"][:].rearrange("p a b -> p (a b)"),
                    )
                    extd2 = nc.dram_tensor(
                        "dbg_ynchunk", [128, 512], dt.bfloat16, kind="ExternalOutput"
                    )
                    nc.sync.dma_start(out=extd2.ap(), in_=yn_sb[:, t0 : t0 + 512])

            th.append((0, f_dma))
            return th

        BIG = 10**9

        def flatten_granules(granules):
            """granules: list of (spec, deadline, gate) where spec is
            ("rms", body, tail) or ("plain", thunks); deadline = git by
            which every item (incl. the deferred rms tail) must be emitted
            (data-producer ordering), gate = git before which items must
            NOT be emitted (data-consumer ordering). Emission order within
            and across granules is preserved; each rms tail is deferred 6
            slots into the following items.

            Returns entries (weight, deadline, gate, fn)."""
            out = []
            pending_tail = []  # (emit_at_index, entry)
            for spec, dl, gate in granules:
                kind, *rest = spec
                if kind == "rms":
                    body, tails = rest
                    items = body
                else:
                    (items,) = rest
                    tails = []
                for w, fn in items:
                    while pending_tail and pending_tail[0][0] <= len(out):
                        out.append(pending_tail.pop(0)[1])
                    out.append((w, dl, gate, fn))
                for off, (tw, tfn) in tails:
                    pending_tail.append((len(out) + off, (tw, dl, gate, tfn)))
                pending_tail.sort(key=lambda e: e[0])
            for _, e in pending_tail:
                out.append(e)
            return out

        # ------------------------------------------------------------------
        # Attention loop
        # ------------------------------------------------------------------

        def qk_tile(b, q0, kt):
            k0 = b * T + kt * 128
            ps_s = pp.tile([128, 2 * JQ], dt.float32, tag="ps")
            nc.tensor.matmul(
                ps_s[:, 0:JQ],
                lhsT=kn_sb[0:64, k0 : k0 + 128],
                rhs=qn_sb[0:64, q0 : q0 + JQ],
                start=True,
                stop=True,
            )
            nc.tensor.matmul(
                ps_s[:, JQ : 2 * JQ],
                lhsT=kn_sb[64:128, k0 : k0 + 128],
                rhs=qn_sb[64:128, q0 : q0 + JQ],
                start=True,
                stop=True,
            )
            return ps_s

        def attn_all(queue1, queue2):
            """queue1 drains during si 0..3 (batch 0), queue2 during si 4..7.
            Weighted pops (1 unit ~ one 250ns PE matmul) with carry-over, so
            an 8-matmul projection thunk is followed by catch-up iterations."""

            def budget_for(git):
                if git < 16:
                    return 5.0
                if git < 64:
                    return 3.5
                if git < 80:
                    return 4.0
                return 3.0

            seq = [(b, jq) for b in range(B) for jq in range(NJQ)]
            ps_cur = qk_tile(0, 0, 0)
            git = 0
            budget = [0.0]
            for si, (b, jq) in enumerate(seq):
                q0 = b * T + jq * JQ
                yA = psp.tile(
                    [HD + 1, JQ], dt.float32, tag="av", bufs=TAG_BUFS["av"], name="yA"
                )
                yB = psp.tile(
                    [HD + 1, JQ], dt.float32, tag="av", bufs=TAG_BUFS["av"], name="yB"
                )
                for kt in range(NKT):
                    se = se_pool.tile([128, 2 * JQ], dt.bfloat16)
                    nc.scalar.activation(
                        out=se[:],
                        in_=ps_cur[:],
                        func=AF.Exp,
                        bias=bias_sb[:, b * NKT + kt : b * NKT + kt + 1],
                        scale=SCALE,
                    )
                    if kt + 1 < NKT:
                        ps_cur = qk_tile(b, q0, kt + 1)
                    elif si + 1 < len(seq):
                        nb, njq = seq[si + 1]
                        ps_cur = qk_tile(nb, nb * T + njq * JQ, 0)
                    # fillers BEFORE the exp-dependent AVs (in-order PE)
                    if git == 64 and queue1:
                        # safety: never drop un-emitted work
                        queue2[:0] = queue1
                        queue1.clear()
                    queue = queue1 if git < 64 else queue2
                    budget[0] += budget_for(git)
                    # forced pops: every entry whose deadline has arrived
                    # (and the FIFO prefix before it) must be emitted NOW —
                    # consumers of their data are about to be emitted.
                    last_due = -1
                    for i, e in enumerate(queue):
                        if e[1] <= git:
                            last_due = i
                    while last_due >= 0:
                        w, _, _, fn = queue.pop(0)
                        budget[0] -= w
                        fn()
                        last_due -= 1
                    # budget pops, respecting not-before gates
                    while queue and queue[0][2] <= git and queue[0][0] <= budget[0]:
                        w, _, _, fn = queue.pop(0)
                        budget[0] -= w
                        fn()
                    budget[0] = max(min(budget[0], 8.0), -16.0)
                    nc.tensor.matmul(
                        yA[:],
                        lhsT=vext[:, 0, b, kt, :],
                        rhs=se[:, 0:JQ],
                        start=(kt == 0),
                        stop=(kt == NKT - 1),
                    )
                    nc.tensor.matmul(
                        yB[:],
                        lhsT=vext[:, 1, b, kt, :],
                        rhs=se[:, JQ : 2 * JQ],
                        start=(kt == 0),
                        stop=(kt == NKT - 1),
                    )
                    git += 1
                # ---- softmax normalize for this (b, jq) ----
                # bf16 copies free the AV psum ring fast (yn is bf16 anyway);
                # the sum row broadcasts via a 1-contraction matmul from
                # partition 64 (ones64 row 64), then reciprocal evacuates
                # the psum. No gpsimd, no gather DMAs.
                yAc = work.tile([128, JQ], dt.float32, tag="yAc", bufs=2)
                yBc = work.tile([128, JQ], dt.float32, tag="yBc", bufs=2)
                nc.vector.tensor_copy(yAc[0:65, :], yA[0:65, :])
                nc.vector.tensor_copy(yBc[0:65, :], yB[0:65, :])
                rs = work.tile([1, 2 * JQ], dt.float32, tag="rs", bufs=2)
                nc.sync.dma_start(out=rs[0:1, 0:JQ], in_=yAc[64:65, :])
                nc.sync.dma_start(out=rs[0:1, JQ : 2 * JQ], in_=yBc[64:65, :])
                rr = work.tile([1, 2 * JQ], dt.float32, tag="rr", bufs=2)
                nc.vector.reciprocal_approx_fast(out=rr[:], in_=rs[:])
                rbA = work.tile([64, JQ], dt.float32, tag="rbA", bufs=2)
                rbB = work.tile([64, JQ], dt.float32, tag="rbB", bufs=2)
                nc.gpsimd.partition_broadcast(rbA[:], rr[0:1, 0:JQ], 64)
                nc.gpsimd.partition_broadcast(rbB[:], rr[0:1, JQ : 2 * JQ], 64)
                nc.vector.tensor_mul(
                    yn_sb[0:64, q0 : q0 + JQ], yAc[0:64, :], rbA[:]
                )
                ynB = work.tile([64, JQ], dt.bfloat16, tag="ynB", bufs=2)
                nc.vector.tensor_mul(ynB[:], yBc[0:64, :], rbB[:])
                nc.sync.dma_start(out=yn_sb[64:128, q0 : q0 + JQ], in_=ynB[:])
            # drain any un-emitted fillers (correctness: every thunk must run)
            for _, _, _, fn in queue1 + queue2:
                fn()
            queue1.clear()
            queue2.clear()

        # ------------------------------------------------------------------
        # Emission schedule
        # ------------------------------------------------------------------
        def rms_g(w_sb, dst_sb, b, g, tag, sib):
            return ("rms",) + rms_granule_thunks(w_sb, dst_sb, b, g, tag, sib)

        def v_g(b, g, tag):
            return ("plain", v_granule_thunks(b, g, tag))

        def op_c(b, ch):
            return ("plain", outproj_chunk_thunks(b, ch))

        # head: kn(b0) complete + qn(b0, jq0) + vext(b0, kt0..7), emitted
        # inline (4-deep psum pipelining across tags proj/op/av/av).
        head = [
            (rms_g(wk_sb, kn_sb, 0, 0, "proj", "av"), BIG, -1),
            (rms_g(wk_sb, kn_sb, 0, 1, "op", "av"), BIG, -1),
            (rms_g(wk_sb, kn_sb, 0, 2, "proj", "av"), BIG, -1),
            (rms_g(wk_sb, kn_sb, 0, 3, "op", "av"), BIG, -1),
            (rms_g(wq_sb, qn_sb, 0, 0, "proj", "av"), BIG, -1),
            (v_g(0, 0, "op"), BIG, -1),
        ]
        for _, _, _, fn in flatten_granules(head):
            fn()

        # fillers for si 0..3: finish b0 (v2, v3, q1-3), then all of b1's
        # k/v/q0. Deadlines force emission before the first consumer's git.
        q1 = flatten_granules(
            [
                (v_g(0, 1, "proj"), 2, -1),
                (v_g(0, 2, "op"), 6, -1),
                (rms_g(wq_sb, qn_sb, 0, 1, "proj", "op"), 9, -1),
                (v_g(0, 3, "op"), 10, -1),
                (rms_g(wq_sb, qn_sb, 0, 2, "proj", "op"), 24, -1),
                (rms_g(wq_sb, qn_sb, 0, 3, "op", "proj"), 40, -1),
                (rms_g(wk_sb, kn_sb, 1, 0, "proj", "op"), 52, -1),
                (rms_g(wk_sb, kn_sb, 1, 1, "op", "proj"), 54, -1),
                (rms_g(wk_sb, kn_sb, 1, 2, "proj", "op"), 56, -1),
                (v_g(1, 0, "op"), 58, -1),
                (rms_g(wk_sb, kn_sb, 1, 3, "proj", "op"), 58, -1),
                (rms_g(wq_sb, qn_sb, 1, 0, "op", "proj"), 58, -1),
                (v_g(1, 1, "proj"), 63, -1),
            ]
        )
        # fillers for si 4..7: late b1 granules + all output projections
        # except the final chunk (tail). Gates keep each outproj chunk from
        # being emitted before the boundary that writes its yn columns.
        q2 = flatten_granules(
            [
                (v_g(1, 2, "proj"), 68, -1),
                (v_g(1, 3, "proj"), 72, -1),
                (rms_g(wq_sb, qn_sb, 1, 1, "proj", "op"), 73, -1),
                (op_c(0, 0), BIG, 16),
                (rms_g(wq_sb, qn_sb, 1, 2, "proj", "op"), 89, -1),
                (op_c(0, 1), BIG, 32),
                (op_c(0, 2), BIG, 48),
                (rms_g(wq_sb, qn_sb, 1, 3, "proj", "op"), 105, -1),
                (op_c(0, 3), BIG, 64),
                (op_c(1, 0), BIG, 80),
                (op_c(1, 1), BIG, 96),
                (op_c(1, 2), BIG, 112),
            ]
        )

        attn_all(q1, q2)
        # fast tail: the final outproj chunk gets the whole now-free PSUM
        # (scores + av rings) so its 8 matmuls pipeline 4-deep, with
        # per-dtile DMAs instead of a staging buffer.
        t0_tail = T + 3 * 512
        TW = NCI * 512
        tail_base = (B * NJQ - 1) * TW
        for dtile in range(NCI):
            if dtile % 2 == 0:
                ps_t = pp.tile([128, 512], dt.float32, tag="ps", name="ps_t")
            else:
                ps_t = psp.tile(
                    [128, 512], dt.float32, tag="av", bufs=TAG_BUFS["av"], name="ps_t"
                )
            nc.tensor.matmul(
                ps_t[:],
                lhsT=wo_sb[:, ts(dtile, 128)],
                rhs=yn_sb[:, t0_tail : t0_tail + 512],
                start=True,
                stop=True,
            )
            obt = work.tile([128, 512], dt.bfloat16, tag="obt", bufs=4, name="obt")
            nc.vector.tensor_copy(obt[:], ps_t[:])
            lo = tail_base + dtile * 512
            nc.sync.dma_start(out=out_ext.ap()[:, lo : lo + 512], in_=obt[:])

        import os

        if bool(int(os.environ.get("BASS_ATTN_DEBUG", "0"))):
            dbg_specs = [
                ("dbg_qn", qn_sb, [128, BT]),
                ("dbg_kn", kn_sb, [128, BT]),
                ("dbg_yn", yn_sb, [128, BT]),
                ("dbg_vext", vext, [128, 2 * B * NKT * (HD + 1)]),
            ]
            for name, srct, shape in dbg_specs:
                extd = nc.dram_tensor(name, shape, dt.bfloat16, kind="ExternalOutput")
                flat = srct[:]
                if len(flat.shape) > 2:
                    flat = flat.rearrange("p a b c d -> p (a b c d)")
                nc.sync.dma_start(out=extd.ap(), in_=flat)

    nc.compile()
    _bacc_mod.get_activation_tables = _orig_tables
    return nc


def _get_nc():
    if "nc" not in _CACHE:
        _CACHE["nc"] = _build_bass()
    return _CACHE["nc"]


def _tile_major(a, width):
    """[C, width] -> [128, NCI*width]: c-tile-major columns, partition = c%128."""
    return np.ascontiguousarray(
        a.reshape(NCI, 128, width).transpose(1, 0, 2).reshape(128, NCI * width)
    )


def _prep_in_maps(x, padding_mask, Wq, Wk, Wv, Wo):
    xf = np.ascontiguousarray(np.asarray(x, dtype=np.float32).reshape(BT, C))
    xt = _tile_major(np.ascontiguousarray(xf.T), BT).astype(BF16)
    # token-group-major: [128, tg, ci, GR] so early granules depend only on
    # early DMA pieces
    xt = np.ascontiguousarray(
        xt.reshape(128, NCI, BT // GR, GR).transpose(0, 2, 1, 3).reshape(128, NCI * BT)
    )
    mb = np.where(
        np.asarray(padding_mask).reshape(BT), np.float32(0.0), np.float32(NEG)
    ).astype(np.float32)
    bias = np.ascontiguousarray(mb.reshape(B * NKT, 128).T)

    in_maps = []
    for i in range(NCORES):
        sl = slice(i * DPC, (i + 1) * DPC)
        in_maps.append(
            {
                "xt": xt,
                "wq": _tile_major(np.ascontiguousarray(Wq[sl, :].T), DPC).astype(BF16),
                "wk": _tile_major(np.ascontiguousarray(Wk[sl, :].T), DPC).astype(BF16),
                "wv": _tile_major(np.ascontiguousarray(Wv[sl, :].T), DPC).astype(BF16),
                "wo": np.ascontiguousarray(Wo[:, sl].T).astype(BF16),
                "bias": bias,
            }
        )
    return in_maps


def _assemble(results):
    total = np.zeros((NCI, 128, BT), dtype=np.float32)
    for r in results:
        # device layout: [128, chunk(8), NCI, 512] -> [NCI, 128, BT]
        o = r["out"].reshape(128, B * NJQ, NCI, 512).astype(np.float32)
        total += o.transpose(2, 0, 1, 3).reshape(NCI, 128, BT)
    return np.ascontiguousarray(total.reshape(C, BT).T).reshape(B, T, C)


def kernel(x, padding_mask, Wq, Wk, Wv, Wo):
    from concourse.bass_utils import run_bass_kernel_spmd

    nc = _get_nc()
    in_maps = _prep_in_maps(x, padding_mask, Wq, Wk, Wv, Wo)
    res = run_bass_kernel_spmd(nc, in_maps, core_ids=list(range(NCORES)))
    return _assemble(res.results)
